# revision 4
# baseline (speedup 1.0000x reference)
import numpy as np
import jax
import jax.numpy as jnp

# ---- fixed configuration (hardcoded; matches the problem spec) ----
B, WIN, N, D, DFF = 32, 96, 64, 64, 256
H = 8
PATCHES = [8, 6, 4, 2]
NUM_EXPERTS, TOPK = 4, 2
EH, KH = 32, 8
LOSS_COEF, LOSS_COEF1 = 0.01, 0.01
M_CORES = 8
BL = B // M_CORES  # local batch per core

BF = jnp.bfloat16
F32 = jnp.float32


def _bf(a):
    return a.astype(BF)


# ---------------- host-side param-only precompute ----------------

def _host_incidence(node_emb: np.ndarray, edge_emb: np.ndarray) -> np.ndarray:
    """A = softmax(where(logits < kth, -inf, logits)) with kth = KH-th largest/row."""
    logits = node_emb @ edge_emb.T                      # [N, EH]
    kth = np.sort(logits, axis=-1)[:, -KH][:, None]
    masked = np.where(logits < kth, -np.inf, logits)
    mx = masked.max(-1, keepdims=True)
    e = np.exp(masked - mx)
    return (e / e.sum(-1, keepdims=True)).astype(np.float32)


def _host_gates(x: np.ndarray, w_gate: np.ndarray):
    """Exact noisy-top-k gating (eval mode) + balance loss, in float32 numpy."""
    logits = np.einsum('bwnd,de->bwne', x, w_gate).astype(np.float32)
    kth = np.sort(logits, axis=-1)[..., -TOPK][..., None]
    masked = np.where(logits < kth, -np.inf, logits)
    mx = masked.max(-1, keepdims=True)
    e = np.exp(masked - mx)
    gates = (e / e.sum(-1, keepdims=True)).astype(np.float32)
    importance = gates.sum(0)
    m = importance.mean()
    v = importance.var(ddof=1)
    balance_loss = v / (m * m + 1e-10) * LOSS_COEF
    return gates, np.float32(balance_loss)


# ---------------- device-side per-shard forward ----------------

def _lin(x, W, b):
    return jnp.matmul(_bf(x), _bf(W), preferred_element_type=F32) + b


def _ln(x, g, b):
    m = x.mean(-1, keepdims=True)
    v = x.var(-1, keepdims=True)
    return (x - m) * jax.lax.rsqrt(v + 1e-5) * g + b


def _attn(q, k, v):
    scale = 1.0 / np.sqrt(q.shape[-1])
    s = jnp.einsum('mlhe,mshe->mhls', _bf(q), _bf(k),
                   preferred_element_type=F32) * scale
    a = jax.nn.softmax(s, axis=-1)
    return jnp.einsum('mhls,mshd->mlhd', _bf(a), _bf(v),
                      preferred_element_type=F32)


def _fuse_layer(x, p, M, ps, bl):
    pn = WIN // ps
    dm = D * ps
    dk, dki = D // H, dm // H
    # hypergraph conv folded: sp_pre = (A @ A.T) applied over nodes
    sp_pre = jnp.einsum('nm,bwmd->bwnd', _bf(M), _bf(x),
                        preferred_element_type=F32)
    sp = _lin(sp_pre, p['sp_W'], p['sp_b'])
    x = sp + x
    new_x = x
    xp = x.reshape(bl, pn, ps, N, D).transpose(0, 1, 3, 4, 2)
    xi = xp.transpose(0, 2, 1, 4, 3)                                   # [bl,N,pn,ps,D]
    q = _lin(p['queries'], p['intra_q_W'], p['intra_q_b']).reshape(N * pn, 1, H, dk)
    q = jnp.broadcast_to(q[None], (bl, N * pn, 1, H, dk)).reshape(bl * N * pn, 1, H, dk)
    kk = _lin(xi, p['intra_k_W'], p['intra_k_b']).reshape(bl * N * pn, ps, H, dk)
    vv = _lin(xi, p['intra_v_W'], p['intra_v_b']).reshape(bl * N * pn, ps, H, dk)
    io = _attn(q, kk, vv).reshape(bl, N, pn, D)
    io = _lin(io, p['intra_o_W'], p['intra_o_b'])
    io = _lin(io.transpose(0, 1, 3, 2), p['intra_lin_W'], p['intra_lin_b'])
    intra_out = io.transpose(0, 3, 1, 2)
    xe = xi.reshape(bl * N, pn, dm)
    q2 = _lin(xe, p['inter_q_W'], p['inter_q_b']).reshape(bl * N, pn, H, dki)
    k2 = _lin(xe, p['inter_k_W'], p['inter_k_b']).reshape(bl * N, pn, H, dki)
    v2 = _lin(xe, p['inter_v_W'], p['inter_v_b']).reshape(bl * N, pn, H, dki)
    eo = _attn(q2, k2, v2).reshape(bl, N, pn, dm)
    eo = _lin(eo, p['inter_o_W'], p['inter_o_b'])
    inter_out = eo.reshape(bl, N, WIN, D).transpose(0, 2, 1, 3)
    out = new_x + intra_out + inter_out
    x1 = _ln(out, p['n1_g'], p['n1_b'])
    y = _lin(jax.nn.gelu(_lin(x1, p['ff1_W'], p['ff1_b']), approximate=False),
             p['ff2_W'], p['ff2_b'])
    return _ln(x1 + y, p['n2_g'], p['n2_b'])


def _shard_forward(x, gates, em):
    expert_params, M_list = em
    bl = x.shape[0]
    acc = x
    for e, (p, M, ps) in enumerate(zip(expert_params, M_list, PATCHES)):
        acc = acc + _fuse_layer(x, p, M, ps, bl) * gates[..., e:e + 1]
    return acc


_pmapped = None
_param_cache = {}  # fingerprint -> device-replicated (eps, M_list)


def _get_pmapped():
    global _pmapped
    if _pmapped is None:
        _pmapped = jax.pmap(_shard_forward, in_axes=(0, 0, 0))
    return _pmapped


def _fingerprint(eps):
    import hashlib
    h = hashlib.sha1()
    for p in eps:
        for k in sorted(p):
            h.update(p[k].tobytes())
    return h.hexdigest()


def _device_params(eps):
    """Replicate folded params across the 8 cores once; cache across calls."""
    fp = _fingerprint(eps)
    if fp not in _param_cache:
        M_list = [(A @ A.T).astype(np.float32)
                  for A in (_host_incidence(p['node_emb'], p['edge_emb']) for p in eps)]
        _param_cache.clear()
        _param_cache[fp] = jax.device_put_replicated((eps, M_list), jax.devices())
    return _param_cache[fp]


def run_device(xs, gs, em_dev):
    """Device-side execution on pre-sharded inputs (used by kernel and test harness)."""
    return _get_pmapped()(xs, gs, em_dev)


def kernel(x, w_gate, expert_params):
    x = np.asarray(x, dtype=np.float32)
    w_gate = np.asarray(w_gate, dtype=np.float32)
    eps = [{k: np.asarray(v, dtype=np.float32) for k, v in p.items()} for p in expert_params]

    # host: param-only incidence matrices + exact gating / losses
    A_list = [_host_incidence(p['node_emb'], p['edge_emb']) for p in eps]
    c_loss = np.float32(sum(float((A * A).mean()) for A in A_list))
    gates, balance_loss = _host_gates(x, w_gate)
    loss = np.float32(balance_loss + c_loss * LOSS_COEF1)

    em_dev = _device_params(eps)
    xs = x.reshape(M_CORES, BL, WIN, N, D)
    gs = gates.reshape(M_CORES, BL, WIN, N, NUM_EXPERTS)

    out = run_device(xs, gs, em_dev)
    out = np.asarray(out).reshape(B, WIN, N, D).astype(np.float32)
    return out, loss


# revision 5
# speedup vs baseline: 1.0297x; 1.0297x over previous
import numpy as np
import jax
import jax.numpy as jnp

# ---- fixed configuration (hardcoded; matches the problem spec) ----
B, WIN, N, D, DFF = 32, 96, 64, 64, 256
H = 8
PATCHES = [8, 6, 4, 2]
NUM_EXPERTS, TOPK = 4, 2
EH, KH = 32, 8
LOSS_COEF, LOSS_COEF1 = 0.01, 0.01
M_CORES = 8
BL = B // M_CORES  # local batch per core

BF = jnp.bfloat16
F32 = jnp.float32


def _bf(a):
    return a.astype(BF)


# ---------------- host-side param-only precompute ----------------

def _host_incidence(node_emb: np.ndarray, edge_emb: np.ndarray) -> np.ndarray:
    """A = softmax(where(logits < kth, -inf, logits)) with kth = KH-th largest/row."""
    logits = node_emb @ edge_emb.T                      # [N, EH]
    kth = np.sort(logits, axis=-1)[:, -KH][:, None]
    masked = np.where(logits < kth, -np.inf, logits)
    mx = masked.max(-1, keepdims=True)
    e = np.exp(masked - mx)
    return (e / e.sum(-1, keepdims=True)).astype(np.float32)


def _host_gates(x: np.ndarray, w_gate: np.ndarray):
    """Exact noisy-top-k gating (eval mode) + balance loss, in float32 numpy."""
    logits = (x.reshape(-1, D) @ w_gate).reshape(B, WIN, N, NUM_EXPERTS).astype(np.float32)
    kth = np.sort(logits, axis=-1)[..., -TOPK][..., None]
    masked = np.where(logits < kth, -np.inf, logits)
    mx = masked.max(-1, keepdims=True)
    e = np.exp(masked - mx)
    gates = (e / e.sum(-1, keepdims=True)).astype(np.float32)
    importance = gates.sum(0)
    m = importance.mean()
    v = importance.var(ddof=1)
    balance_loss = v / (m * m + 1e-10) * LOSS_COEF
    return gates, np.float32(balance_loss)


# ---------------- device-side per-shard forward ----------------

def _lin(x, W, b):
    return jnp.matmul(_bf(x), _bf(W), preferred_element_type=F32) + b


def _ln(x, g, b):
    m = x.mean(-1, keepdims=True)
    v = x.var(-1, keepdims=True)
    return (x - m) * jax.lax.rsqrt(v + 1e-5) * g + b


def _attn(q, k, v):
    scale = 1.0 / np.sqrt(q.shape[-1])
    s = jnp.einsum('mlhe,mshe->mhls', _bf(q), _bf(k),
                   preferred_element_type=F32) * scale
    a = jax.nn.softmax(s, axis=-1)
    return jnp.einsum('mhls,mshd->mlhd', _bf(a), _bf(v),
                      preferred_element_type=F32)


def _fuse_layer(x, p, M, ps, bl):
    pn = WIN // ps
    dm = D * ps
    dk, dki = D // H, dm // H
    # hypergraph conv folded: sp_pre = (A @ A.T) applied over nodes
    sp_pre = jnp.einsum('nm,bwmd->bwnd', _bf(M), _bf(x),
                        preferred_element_type=F32)
    sp = _lin(sp_pre, p['sp_W'], p['sp_b'])
    x = sp + x
    new_x = x
    xp = x.reshape(bl, pn, ps, N, D).transpose(0, 1, 3, 4, 2)
    xi = xp.transpose(0, 2, 1, 4, 3)                                   # [bl,N,pn,ps,D]
    q = _lin(p['queries'], p['intra_q_W'], p['intra_q_b']).reshape(N * pn, 1, H, dk)
    q = jnp.broadcast_to(q[None], (bl, N * pn, 1, H, dk)).reshape(bl * N * pn, 1, H, dk)
    kk = _lin(xi, p['intra_k_W'], p['intra_k_b']).reshape(bl * N * pn, ps, H, dk)
    vv = _lin(xi, p['intra_v_W'], p['intra_v_b']).reshape(bl * N * pn, ps, H, dk)
    io = _attn(q, kk, vv).reshape(bl, N, pn, D)
    io = _lin(io, p['intra_o_W'], p['intra_o_b'])
    io = _lin(io.transpose(0, 1, 3, 2), p['intra_lin_W'], p['intra_lin_b'])
    intra_out = io.transpose(0, 3, 1, 2)
    xe = xi.reshape(bl * N, pn, dm)
    q2 = _lin(xe, p['inter_q_W'], p['inter_q_b']).reshape(bl * N, pn, H, dki)
    k2 = _lin(xe, p['inter_k_W'], p['inter_k_b']).reshape(bl * N, pn, H, dki)
    v2 = _lin(xe, p['inter_v_W'], p['inter_v_b']).reshape(bl * N, pn, H, dki)
    eo = _attn(q2, k2, v2).reshape(bl, N, pn, dm)
    eo = _lin(eo, p['inter_o_W'], p['inter_o_b'])
    inter_out = eo.reshape(bl, N, WIN, D).transpose(0, 2, 1, 3)
    out = new_x + intra_out + inter_out
    x1 = _ln(out, p['n1_g'], p['n1_b'])
    y = _lin(jax.nn.gelu(_lin(x1, p['ff1_W'], p['ff1_b']), approximate=False),
             p['ff2_W'], p['ff2_b'])
    return _ln(x1 + y, p['n2_g'], p['n2_b'])


def _shard_forward(x, gates, em):
    expert_params, M_list = em
    bl = x.shape[0]
    acc = x
    for e, (p, M, ps) in enumerate(zip(expert_params, M_list, PATCHES)):
        acc = acc + _fuse_layer(x, p, M, ps, bl) * gates[..., e:e + 1]
    return acc


_pmapped = None
_param_cache = {}  # fingerprint -> device-replicated (eps, M_list)


def _get_pmapped():
    global _pmapped
    if _pmapped is None:
        _pmapped = jax.pmap(_shard_forward, in_axes=(0, 0, 0))
    return _pmapped


def _fingerprint(eps):
    import hashlib
    h = hashlib.sha1()
    for p in eps:
        for k in sorted(p):
            h.update(p[k].tobytes())
    return h.hexdigest()


def _device_params(eps):
    """Replicate folded params across the 8 cores once; cache across calls."""
    fp = _fingerprint(eps)
    if fp not in _param_cache:
        M_list = [(A @ A.T).astype(np.float32)
                  for A in (_host_incidence(p['node_emb'], p['edge_emb']) for p in eps)]
        _param_cache.clear()
        _param_cache[fp] = jax.device_put_replicated((eps, M_list), jax.devices())
    return _param_cache[fp]


def run_device(xs, gs, em_dev):
    """Device-side execution on pre-sharded inputs (used by kernel and test harness)."""
    return _get_pmapped()(xs, gs, em_dev)


def kernel(x, w_gate, expert_params):
    x = np.asarray(x, dtype=np.float32)
    w_gate = np.asarray(w_gate, dtype=np.float32)
    eps = [{k: np.asarray(v, dtype=np.float32) for k, v in p.items()} for p in expert_params]

    # host: param-only incidence matrices + exact gating / losses
    A_list = [_host_incidence(p['node_emb'], p['edge_emb']) for p in eps]
    c_loss = np.float32(sum(float((A * A).mean()) for A in A_list))
    gates, balance_loss = _host_gates(x, w_gate)
    loss = np.float32(balance_loss + c_loss * LOSS_COEF1)

    em_dev = _device_params(eps)
    xs = x.reshape(M_CORES, BL, WIN, N, D)
    gs = gates.reshape(M_CORES, BL, WIN, N, NUM_EXPERTS)

    out = run_device(xs, gs, em_dev)
    out = np.asarray(out).reshape(B, WIN, N, D).astype(np.float32)
    return out, loss
